# revision 2
# baseline (speedup 1.0000x reference)
"""MoE routed expert matmul on 8 Trainium2 NeuronCores.

Problem: out[n] = input[n] @ w[inds[n]] + b[inds[n]]
  input [262144, 32] f32, inds [262144] i32 (1024 experts), w [1024, 32, 32], b [1024, 1, 32]

Strategy (expert-sharded, quad-packed; host does routing/layout only):
  * Host ranks the 1024 experts by token count (descending) and forms 256
    global quads of 4 rank-consecutive experts; quad q goes to core q%8,
    slot j=q//8.  All 32 ranks feeding slot j are consecutive, so one
    shared slot width P[j] = count of rank 32j pads every core's slot to
    the same size (SPMD-identical program) with ~1-2% padding.
  * Per quad, the 4 experts' [32, 32] weight blocks sit on the diagonal of
    a [128, 128] block-diagonal lhsT (off-diagonal zeros).  One fp16
    matmul then serves 4 tokens per streamed column: band r of rhs column
    t holds the features of token t of expert r, and PSUM partitions
    32r..32r+32 of that column receive its 32 outputs.  This packs the
    whole core's work into 32 matmuls of ~260 columns instead of 128.
  * The block-diagonal weights are built on device: memset a [128, 4096]
    fp16 tile, then strided band copies from the compact [128, 1024]
    weight load (4 small copies for group 0, 4 batched copies for the
    rest), keeping the extra 0.75 MB off the DMA rings.
  * Groups of 4 quads share a 4-bank [128, 2048] PSUM tile (2 in flight).
    PSUM->SBUF copy + per-quad bias and fp32->fp16 conversion alternate
    between Vector (tensor_scalar_add) and Scalar (activation Identity).
    fp16 output halves store traffic; rel. error stays ~5e-4.
  * Loads ride the SP HWDGE ring; stores alternate GpSimd SWDGE / Scalar
    HWDGE so descriptor generation overlaps.  All DMA serializes at
    360 GB/s, so fp16 everywhere puts the kernel near the memory
    roofline: ~4.6 MB/core ~= 12.7 us of DMA busy.
  * Host scatters the sorted fp16 outputs back to token order in fp32.

Layouts (core k, slot j = 0..31, band r = 0..3, expert e(k,j,r) =
rank_order[4*(8j+k)+r], X[j] = column offset, P[j] = slot width):
  xt [128, TOTC]  xt[32r+i, X[j]+t] = x[token t of e(k,j,r), feat i]  (fp16)
  wq [128, 1024]  wq[32r+i, 32j+o]  = w[e(k,j,r), i, o]               (fp16)
  bp [128, 32]    bp[32r+o, j]      = b[e(k,j,r), 0, o]               (f32)
  ot [128, TOTC]  ot[32r+o, X[j]+t] = out[token t of e(k,j,r), o]     (fp16)
"""

import numpy as np

import concourse.bass as bass
import concourse.mybir as mybir
import concourse.tile as tile
from concourse import bacc
from concourse.bass_utils import run_bass_kernel_spmd

N_TOK = 262144
E = 1024
F = 32
O = 32
NCORES = 8
SLOTS = 32  # quads per core
GROUPS = 8  # groups of 4 quads
F32 = mybir.dt.float32
F16 = mybir.dt.float16

N_WARM = 14  # PE clock-ramp warmup matmuls

_programs: dict[tuple, "bacc.Bacc"] = {}


class _CapacityOverflow(Exception):
    """A single expert got >512 tokens (~16 sigma out for uniform routing at
    256 tokens/expert).  Handled by a host fallback so kernel() still
    returns a correct result."""


def _plan(counts):
    """Slot widths P[j] (shared across cores), offsets X, group widths W."""
    rank_order = np.argsort(-counts, kind="stable")  # expert ids, desc count
    P = counts[rank_order[0::32]].astype(np.int64)  # [32] max of each window
    P = np.maximum(P, 16)
    if P.max() > 512:
        raise _CapacityOverflow(int(P.max()))
    X = np.zeros(SLOTS, dtype=np.int64)
    np.cumsum(P[:-1], out=X[1:])
    W = P.reshape(GROUPS, 4).sum(axis=1)  # [8] group widths
    return rank_order, P, X, W, int(P.sum())


def _build(P, X, W, TOTC) -> "bacc.Bacc":
    nc = bacc.Bacc("TRN2", target_bir_lowering=False, debug=False, num_devices=NCORES)
    xt = nc.declare_dram_parameter("xt", [128, TOTC], F16, isOutput=False)
    wq = nc.declare_dram_parameter("wq", [128, SLOTS * O], F16, isOutput=False)
    bp = nc.declare_dram_parameter("bp", [128, SLOTS], F32, isOutput=False)
    ot = nc.declare_dram_parameter("ot", [128, TOTC], F16, isOutput=True)

    with tile.TileContext(nc) as tc:
        with (
            tc.tile_pool(name="w", bufs=1) as w_pool,
            tc.tile_pool(name="xt", bufs=3) as xt_pool,
            tc.tile_pool(name="out", bufs=3) as out_pool,
            tc.tile_pool(name="psum", bufs=2, space="PSUM") as psum_pool,
        ):
            wq_t = w_pool.tile([128, SLOTS * O], F16)
            nc.sync.dma_start(out=wq_t[:], in_=wq[:])
            bp_t = w_pool.tile([128, SLOTS], F32)
            nc.sync.dma_start(out=bp_t[:], in_=bp[:])

            # Block-diagonal lhsT store: wbd[32r+i, 128j+32r+o] = wq[32r+i, 32j+o],
            # zero off-diagonal.  Zeroing is split so group 0 unblocks first and
            # the bulk lands on the otherwise-idle GpSimd engine.
            wbd = w_pool.tile([128, SLOTS * 128], F16)
            nc.vector.memset(wbd[:, :512], 0.0)
            nc.vector.memset(wbd[:, 512:2304], 0.0)
            nc.gpsimd.memset(wbd[:, 2304:], 0.0)

            # PE warm-up on a zeroed tile: keeps the clock ramp (HAM) going
            # while the first loads are in flight; PSUM output never read.
            warm = w_pool.tile([32, 128], F16)
            nc.vector.memset(warm[:], 0.0)
            warm_ps = psum_pool.tile([128, 128], F32, space="PSUM", name="wps", tag="ps")
            for _ in range(N_WARM):
                nc.tensor.matmul(
                    out=warm_ps[0:32, :],
                    lhsT=warm[:, 0:32],
                    rhs=warm[:, :],
                    start=True,
                    stop=True,
                    tile_position=(0, 0),
                )

            # Band copies into the block-diagonal store: group 0 alone (small,
            # unblocks the first matmuls), then groups 1-7 batched.
            for r in range(4):
                src = wq_t[32 * r : 32 * r + 32, : 4 * O].rearrange(
                    "p (j o) -> p j o", o=O
                )
                dst = wbd[32 * r : 32 * r + 32, : 4 * 128].rearrange(
                    "p (j o) -> p j o", o=128
                )[:, :, 32 * r : 32 * r + 32]
                nc.vector.tensor_copy(dst, src)
            for r in range(4):
                src = wq_t[32 * r : 32 * r + 32, 4 * O :].rearrange(
                    "p (j o) -> p j o", o=O
                )
                dst = wbd[32 * r : 32 * r + 32, 4 * 128 :].rearrange(
                    "p (j o) -> p j o", o=128
                )[:, :, 32 * r : 32 * r + 32]
                nc.vector.tensor_copy(dst, src)

            for g in range(GROUPS):
                Wg, Xg = int(W[g]), int(X[4 * g])
                xt_t = xt_pool.tile([128, Wg], F16, name="xt_t", tag="xt_t")
                nc.sync.dma_start(out=xt_t[:], in_=xt[:, Xg : Xg + Wg])
                o_t = out_pool.tile([128, Wg], F16, name="o_t", tag="o_t")
                psum = psum_pool.tile([128, 2048], F32, space="PSUM", name="ps", tag="ps")

                xo = 0
                for qi in range(4):
                    j = 4 * g + qi
                    Pj = int(P[j])
                    nc.tensor.matmul(
                        out=psum[:, 512 * qi : 512 * qi + Pj],
                        lhsT=wbd[:, 128 * j : 128 * j + 128],
                        rhs=xt_t[:, xo : xo + Pj],
                        start=True,
                        stop=True,
                    )
                    xo += Pj

                xo = 0
                for qi in range(4):
                    j = 4 * g + qi
                    Pj = int(P[j])
                    bias_ap = bp_t[:, j : j + 1]
                    if g % 2 == 0:
                        nc.vector.tensor_scalar_add(
                            o_t[:, xo : xo + Pj],
                            psum[:, 512 * qi : 512 * qi + Pj],
                            bias_ap,
                        )
                    else:
                        nc.scalar.activation(
                            o_t[:, xo : xo + Pj],
                            psum[:, 512 * qi : 512 * qi + Pj],
                            mybir.ActivationFunctionType.Identity,
                            bias=bias_ap,
                            scale=1.0,
                        )
                    xo += Pj

                if g % 2 == 0:
                    nc.gpsimd.dma_start(out=ot[:, Xg : Xg + Wg], in_=o_t[:])
                else:
                    nc.scalar.dma_start(out=ot[:, Xg : Xg + Wg], in_=o_t[:])

    nc.compile()
    return nc


def _pack(x, inds, w, b):
    """Host-side routing: rank experts, build per-core device arrays."""
    counts = np.bincount(inds, minlength=E)
    rank_order, P, X, W, TOTC = _plan(counts)

    rank_of = np.empty(E, dtype=np.int64)
    rank_of[rank_order] = np.arange(E)
    q_glob = rank_of // 4
    r_all = rank_of % 4  # [E] band of each expert
    k_all = q_glob % NCORES  # [E] core of each expert
    j_all = q_glob // NCORES  # [E] slot of each expert

    order = np.argsort(inds, kind="stable")
    sorted_inds = inds[order]
    starts = np.zeros(E, dtype=np.int64)
    np.cumsum(counts[:-1], out=starts[1:])
    slot_tok = np.arange(N_TOK, dtype=np.int64) - starts[sorted_inds]

    k_tok = k_all[sorted_inds]
    r_tok = r_all[sorted_inds]
    col_tok = X[j_all[sorted_inds]] + slot_tok

    xt_all = np.zeros((NCORES, 4, F, TOTC), dtype=np.float16)
    xt_all[k_tok, r_tok, :, col_tok] = x[order].astype(np.float16)
    xt = xt_all.reshape(NCORES, 128, TOTC)

    e_kjr = rank_order.reshape(SLOTS, NCORES, 4)  # [j, k, r] -> expert id
    # wq[k, 32r+i, 32j+o] = w[e(k,j,r), i, o]
    wq = np.ascontiguousarray(
        w.astype(np.float16)[e_kjr].transpose(1, 2, 3, 0, 4)
    ).reshape(NCORES, 128, SLOTS * O)
    # bp[k, 32r+o, j] = b[e(k,j,r), 0, o]
    bp = np.ascontiguousarray(
        b[:, 0, :].astype(np.float32)[e_kjr].transpose(1, 2, 3, 0)
    ).reshape(NCORES, 128, SLOTS)

    plan = (P, X, W, TOTC)
    return plan, order, (k_tok, r_tok, col_tok), xt, wq, bp


def _unpack(results, tok_addr, order):
    k_tok, r_tok, col_tok = tok_addr
    ot = np.stack([results[k]["ot"] for k in range(NCORES)])  # [k, 128, TOTC]
    ot4 = ot.reshape(NCORES, 4, O, -1)  # [k, r, o, col]
    out = np.empty((N_TOK, O), dtype=np.float32)
    out[order] = ot4[k_tok, r_tok, :, col_tok]
    return out


def kernel(input, inds, w, b):
    x = np.ascontiguousarray(np.asarray(input, dtype=np.float32))
    inds = np.asarray(inds, dtype=np.int32)
    w = np.ascontiguousarray(np.asarray(w, dtype=np.float32))
    b = np.ascontiguousarray(np.asarray(b, dtype=np.float32))
    assert x.shape == (N_TOK, F) and inds.shape == (N_TOK,)
    assert w.shape == (E, F, O) and b.shape == (E, 1, O)

    try:
        plan, order, tok_addr, xt, wq, bp = _pack(x, inds, w, b)
    except _CapacityOverflow:
        return (np.einsum("ni,nio->no", x, w[inds]) + b[inds, 0]).astype(np.float32)
    P, X, W, TOTC = plan

    key = P.tobytes()
    nc = _programs.get(key)
    if nc is None:
        nc = _build(P, X, W, TOTC)
        _programs[key] = nc

    in_maps = [{"xt": xt[k], "wq": wq[k], "bp": bp[k]} for k in range(NCORES)]
    res = run_bass_kernel_spmd(nc, in_maps, list(range(NCORES)))

    return _unpack(res.results, tok_addr, order)


def last_program():
    """The most recently compiled Bass program (for profiling in test.py)."""
    return next(iter(_programs.values())) if _programs else None
